# revision 23
# baseline (speedup 1.0000x reference)
"""Graphormer encoder layer on 8 trn2 NeuronCores.

Sharding: batch (4) x query-half (2) -> 8 cores, no collectives.
Core c handles batch b=c//2, query rows [q0, q0+448) with q0=(c%2)*448.
Only the first 896 sequence positions are computed (last 128 are padding:
keys are masked out and the reference zeroes those output rows, which the
host does during assembly).

Structure (single pass, PE kept dense):
- QKV and attention are software-pipelined per head-pair m: scores/exp for
  pair m run interleaved with attention-V of pair m-1 and the QKV chains of
  pair m+1, so the PE stays fed while the Activation engine does the exps.
- attn_bias enters softmax as exp(s+b) = exp(s)*exp(b); exp(b) is
  precomputed on the host, so no bias-add matmul on the PE.
- attention-V uses a [V_h | ones] stationary so one matmul pass yields both
  the weighted values (partitions 0-63) and the softmax denominator
  replicated on partitions 64-127.
- proj inputs and the first half of w1 are prefetched into the DMA-idle
  window during attention; the rest of w1 plus w2 stream during phase D/E1.
- FFN1 runs in two query groups so it starts right after the first two
  transposes; FFN2 runs tile-serial so LayerNorm2 of tile i overlaps the
  matmuls of tile i+1; ffn_b2+ln1_b ride an extra K=1 matmul row.
- ln1_g is folded into w1 on the host; ln2_g/ln2_b are applied on the host
  during assembly (both exact rewrites of the reference math).

Numerics: fp32 PSUM accumulation everywhere. QKV, proj, and FFN2 matmuls
run in fp8e4m3 with DoubleRow perf mode (weights pre-scaled x16/x64 on the
host, rescaled on psum read-out); attention scores/attention-V and FFN1
stay bf16. Measured end-to-end rel err 0.0168 vs the fp32 reference
(softmax and LayerNorm wash out the QKV/proj quantization; FFN2 fp8 is the
dominant, deliberate error term within the 2e-2 budget).
"""

import sys

sys.path.insert(0, "/opt/trn_rl_repo")

import numpy as np
import ml_dtypes

import concourse.bass as bass
import concourse.tile as tile
from concourse import bacc, mybir
from concourse.bass_utils import run_bass_kernel_spmd
from concourse.masks import make_identity

BF16 = mybir.dt.bfloat16
F32 = mybir.dt.float32
FP8 = mybir.dt.float8e4
AF = mybir.ActivationFunctionType
ALU = mybir.AluOpType
DR = mybir.MatmulPerfMode.DoubleRow
# fp8 weight pre-scales (host multiplies, kernel divides on psum read-out)
SQ8, SK8, SV8, SP8, SW2 = 64.0, 16.0, 16.0, 16.0, 16.0

B, S, H, NH, F = 4, 1024, 1024, 16, 4096
HD = H // NH          # 64
PAD = 128
SV = S - PAD          # 896 valid rows
R = SV // 2           # 448 query rows per core
NKT = SV // 128       # 7 k tiles
NHC = H // 128        # 8 chunks of H
NFT = F // 128        # 32 tiles of F
NP = NH // 2          # 8 head pairs
EPS = 1e-5
# q tiles within the 448 rows (last one ragged)
QT = [(0, 128), (128, 128), (256, 128), (384, 64)]


def free_bcast(ap2d, reps):
    """Insert a stride-0 dim after the partition dim: [P, W] -> [P, reps, W]."""
    return bass.AP(
        tensor=ap2d.tensor,
        offset=ap2d.offset,
        ap=[ap2d.ap[0], [0, reps]] + list(ap2d.ap[1:]),
    )


def bcast_row(dram_ap, offset_elems, row_len, nparts=128):
    """AP reading row_len dram elems replicated across nparts partitions."""
    base = dram_ap.ap()
    return bass.AP(
        tensor=base.tensor,
        offset=base.offset + offset_elems,
        ap=[[0, nparts], [1, row_len]],
    )


def build_program():
    nc = bacc.Bacc("TRN2", target_bir_lowering=False, debug=False)

    d_xT = nc.dram_tensor("xT", [H, SV], FP8, kind="ExternalInput")
    d_xq = nc.dram_tensor("xq", [R, H], F32, kind="ExternalInput")
    d_expbT = nc.dram_tensor("expbT", [SV, R], BF16, kind="ExternalInput")
    d_qkvw = nc.dram_tensor("qkvw", [H, 3 * H], FP8, kind="ExternalInput")
    d_qkb = nc.dram_tensor("qkb", [128, 16], F32, kind="ExternalInput")
    d_vb = nc.dram_tensor("vb", [1, H], F32, kind="ExternalInput")
    d_projw = nc.dram_tensor("projw", [H, H], FP8, kind="ExternalInput")
    d_w1 = nc.dram_tensor("w1", [H, F], BF16, kind="ExternalInput")
    d_b1t = nc.dram_tensor("b1t", [128, NFT], F32, kind="ExternalInput")
    d_w2 = nc.dram_tensor("w2", [F, H], FP8, kind="ExternalInput")
    d_b2row = nc.dram_tensor("b2row", [1, H], BF16, kind="ExternalInput")
    # rows: ln1_g, ln1_b, ln2_g, ln2_b
    d_lnp = nc.dram_tensor("lnp", [4, H], F32, kind="ExternalInput")
    d_out = nc.dram_tensor("out", [R, H], F32, kind="ExternalOutput")

    with tile.TileContext(nc) as tc:
        with (
            tc.tile_pool(name="const", bufs=1) as const,
            tc.tile_pool(name="gD", bufs=1) as gD,   # attnT + proj inputs
            tc.tile_pool(name="gW1a", bufs=1) as gW1a,  # first half of w1,
            # prefetched into the DMA-idle window during attention
        ):
            ident = const.tile([128, 128], F32)
            make_identity(nc, ident)
            eps_t = const.tile([128, 1], F32, tag="eps")
            nc.vector.memset(eps_t[:], EPS)
            ones1 = const.tile([1, 128], BF16, tag="ones1")
            nc.vector.memset(ones1[:], 1.0)
            qkb = const.tile([128, 16], F32, tag="qkb")
            nc.sync.dma_start(qkb[:], d_qkb.ap())
            b1t = const.tile([128, NFT], F32, tag="b1t")
            vb_bc = const.tile([128, H], F32, tag="vb")
            b2row = const.tile([1, H], BF16, tag="b2row")
            ln1g = const.tile([128, H], F32, tag="ln1g")

            attnT = gD.tile([128, NHC, R], FP8, tag="attnT")

            # -------- interleaved QKV (B) + attention (C) --------
            with (
                tc.tile_pool(name="gBC", bufs=1) as gBC,
                tc.tile_pool(name="pair", bufs=2) as pair,
                tc.tile_pool(name="psA", bufs=2, space="PSUM") as psA,
                tc.tile_pool(name="psS", bufs=2, space="PSUM") as psS,
                tc.tile_pool(name="psV", bufs=2, space="PSUM") as psV,
            ):
                qkvw_sb = gBC.tile([128, NHC, 3 * H], FP8, tag="qkvw")
                xT_sb = gBC.tile([128, NHC, SV], FP8, tag="xT")
                expb_sb = gBC.tile([128, NKT, R], BF16, tag="expb")

                def qkvw_cols(c0, w):
                    """DMA qkvw columns [c0, c0+w) into qkvw_sb (all kc)."""
                    nc.sync.dma_start(
                        qkvw_sb[:, :, c0 : c0 + w],
                        d_qkvw.ap()[:, c0 : c0 + w].rearrange(
                            "(kc p) c -> p kc c", p=128
                        ),
                    )

                # ordered so pair-0 chains start early
                qkvw_cols(0, 512)
                for kc in range(8):
                    nc.sync.dma_start(
                        xT_sb[:, kc, :], d_xT.ap()[kc * 128 : (kc + 1) * 128, :]
                    )
                qkvw_cols(H, 512)
                qkvw_cols(2 * H, 512)
                nc.sync.dma_start(vb_bc[:], bcast_row(d_vb, 0, H))
                nc.sync.dma_start(
                    expb_sb[:], d_expbT.ap().rearrange("(t p) q -> p t q", p=128)
                )
                qkvw_cols(512, 512)
                qkvw_cols(H + 512, 512)
                qkvw_cols(2 * H + 512, 512)

                # prefetch phase-D inputs (issued now; lands during attention)
                xq_sb = gD.tile([128, 4, H], F32, tag="xq")
                for i, (o, sz) in enumerate(QT):
                    nc.sync.dma_start(xq_sb[:sz, i, :], d_xq.ap()[o : o + sz, :])
                projw_sb = gD.tile([128, NHC, H], FP8, tag="projw")
                for kc in range(NHC):
                    nc.sync.dma_start(
                        projw_sb[:, kc, :],
                        d_projw.ap()[kc * 128 : (kc + 1) * 128, :],
                    )
                nc.sync.dma_start(ln1g[:], bcast_row(d_lnp, 0, H))
                nc.sync.dma_start(b1t[:], d_b1t.ap())
                nc.sync.dma_start(b2row[:], d_b2row.ap())
                w1a_sb = gW1a.tile([128, NHC, F // 2], BF16, tag="w1a")
                for c in range(4):
                    nc.sync.dma_start(
                        w1a_sb[:, :, c * 512 : (c + 1) * 512],
                        d_w1.ap()[:, c * 512 : (c + 1) * 512].rearrange(
                            "(kc p) f -> p kc f", p=128
                        ),
                    )

                prev = None  # (m, E0, E1, vnat) awaiting attention-V

                def attn_v_j(m, E, vnat, j):
                    """attention-V + divide for (pair m, head slot j)."""
                    pv = psV.tile([128, 512], F32, tag="pv", name="pv")
                    for t in range(NKT):
                        nc.tensor.matmul(
                            pv[:, :R],
                            vnat[:, t, j, :],
                            E[:, t, :],
                            start=(t == 0),
                            stop=(t == NKT - 1),
                        )
                    rec = pair.tile([128, R], F32, tag=f"rec{j}",
                                    name=f"rec{j}")
                    nc.vector.reciprocal(rec[64:128, :], pv[64:128, :R])
                    nc.vector.tensor_tensor(
                        out=attnT[64 * j : 64 * j + 64, m, :],
                        in0=pv[0:64, :R],
                        in1=rec[64:128, :],
                        op=ALU.mult,
                    )

                def qkv_pair(m):
                    """QKV chains for pair m; returns (qTm, kTm, vnat)."""
                    qTm = pair.tile([128, R], BF16, tag="qTm", name="qTm")
                    kTm = pair.tile([128, SV], BF16, tag="kTm", name="kTm")
                    vnat = pair.tile([128, NKT, 2, 128], BF16, tag="vnat",
                                     name="vnat")
                    # ones blocks for the fused attention-V row sums
                    nc.gpsimd.memset(vnat[:, :, :, 64:128], 1.0)

                    ps = psA.tile([128, 512], F32, tag="qkv", name="ps")
                    for k2 in range(NHC // 2):
                        nc.tensor.matmul(
                            ps[:, :R],
                            qkvw_sb[:, 2 * k2 : 2 * k2 + 2,
                                    m * 128 : (m + 1) * 128],
                            xT_sb[:, 2 * k2 : 2 * k2 + 2, 0:R],
                            start=(k2 == 0),
                            stop=(k2 == NHC // 2 - 1),
                            perf_mode=DR,
                        )
                    nc.vector.tensor_scalar(
                        out=qTm[:], in0=ps[:, :R],
                        scalar1=1.0 / SQ8, scalar2=qkb[:, m : m + 1],
                        op0=ALU.mult, op1=ALU.add,
                    )
                    for n in range(2):
                        ps = psA.tile([128, 512], F32, tag="qkv", name="ps")
                        for k2 in range(NHC // 2):
                            nc.tensor.matmul(
                                ps[:, :R],
                                qkvw_sb[:, 2 * k2 : 2 * k2 + 2,
                                        H + m * 128 : H + (m + 1) * 128],
                                xT_sb[:, 2 * k2 : 2 * k2 + 2,
                                      n * R : (n + 1) * R],
                                start=(k2 == 0),
                                stop=(k2 == NHC // 2 - 1),
                                perf_mode=DR,
                            )
                        if n == 0:
                            nc.vector.tensor_scalar(
                                out=kTm[:, n * R : (n + 1) * R], in0=ps[:, :R],
                                scalar1=1.0 / SK8, scalar2=qkb[:, 8 + m : 9 + m],
                                op0=ALU.mult, op1=ALU.add,
                            )
                        else:
                            nc.scalar.activation(
                                kTm[:, n * R : (n + 1) * R], ps[:, :R],
                                AF.Identity, scale=1.0 / SK8,
                                bias=qkb[:, 8 + m : 9 + m],
                            )
                    for t0 in range(0, NKT, 4):
                        tw = min(4, NKT - t0)
                        ps = psA.tile([128, 4, 128], F32, tag="qkv", name="ps")
                        for t in range(t0, t0 + tw):
                            for k2 in range(NHC // 2):
                                nc.tensor.matmul(
                                    ps[:, t - t0, :],
                                    xT_sb[:, 2 * k2 : 2 * k2 + 2,
                                          t * 128 : (t + 1) * 128],
                                    qkvw_sb[:, 2 * k2 : 2 * k2 + 2,
                                            2 * H + m * 128 : 2 * H + (m + 1) * 128],
                                    start=(k2 == 0),
                                    stop=(k2 == NHC // 2 - 1),
                                    perf_mode=DR,
                                )
                        nc.vector.scalar_tensor_tensor(
                            out=vnat[:, t0 : t0 + tw, 0:2, 0:64],
                            in0=ps[:, 0:tw, :],
                            scalar=1.0 / SV8,
                            in1=free_bcast(vb_bc[:, m * 128 : (m + 1) * 128], tw),
                            op0=ALU.mult,
                            op1=ALU.add,
                        )
                    return qTm, kTm, vnat

                cur = qkv_pair(0)
                for m in range(NP):
                    qTm, kTm, vnat = cur
                    # scores + exp for pair m; the previous pair's
                    # attention-V and the NEXT pair's QKV are interleaved
                    # to keep the PE fed during the exp waits
                    Es = [
                        pair.tile([128, NKT, R], BF16, tag="E0", name="E0"),
                        pair.tile([128, NKT, R], BF16, tag="E1", name="E1"),
                    ]

                    def sc_batch(j, t0):
                        po = 64 * j
                        tw = min(2, NKT - t0)
                        sc = psS.tile([128, 2, 512], F32, tag="sc", name="sc")
                        for t in range(t0, t0 + tw):
                            nc.tensor.matmul(
                                sc[:, t - t0, :R],
                                kTm[po : po + 64, t * 128 : (t + 1) * 128],
                                qTm[po : po + 64, :],
                                start=True,
                                stop=True,
                            )
                        nc.scalar.activation(
                            Es[j][:, t0 : t0 + tw, :], sc[:, 0:tw, 0:R], AF.Exp
                        )

                    def expb_mult(j):
                        # fold in exp(attn_bias) (host-precomputed)
                        nc.vector.tensor_tensor(
                            out=Es[j][:, :, :], in0=Es[j][:, :, :],
                            in1=expb_sb[:, :, :], op=ALU.mult,
                        )

                    sc_batch(0, 0)
                    sc_batch(0, 2)
                    if prev is not None:
                        attn_v_j(prev[0], prev[1][0], prev[2], 0)
                    sc_batch(0, 4)
                    sc_batch(0, 6)
                    expb_mult(0)
                    if prev is not None:
                        attn_v_j(prev[0], prev[1][1], prev[2], 1)
                    if m < NP - 1:
                        cur = qkv_pair(m + 1)
                    sc_batch(1, 0)
                    sc_batch(1, 2)
                    sc_batch(1, 4)
                    sc_batch(1, 6)
                    expb_mult(1)

                    prev = (m, Es, vnat)

                attn_v_j(prev[0], prev[1][0], prev[2], 0)
                attn_v_j(prev[0], prev[1][1], prev[2], 1)

            # -------- proj + LN1 + transpose (D), FFN (E) --------
            with (
                tc.tile_pool(name="gW", bufs=1) as gW,
                tc.tile_pool(name="lpool", bufs=2) as lpool,
            ):
                y_sb = gW.tile([128, 4, H], F32, tag="y")
                yT = gW.tile([128, NHC, R], BF16, tag="yT")
                hT = gW.tile([128, NFT, R], FP8, tag="hT")
                w1b_sb = gW.tile([128, NHC, F // 2], BF16, tag="w1b")
                w2_sb = gW.tile([128, NFT, H], FP8, tag="w2")
                for c in range(4):
                    nc.sync.dma_start(
                        w1b_sb[:, :, c * 512 : (c + 1) * 512],
                        d_w1.ap()[:, 2048 + c * 512 : 2048 + (c + 1) * 512]
                        .rearrange("(kc p) f -> p kc f", p=128),
                    )
                for fc in range(NFT):
                    nc.sync.dma_start(
                        w2_sb[:, fc, :], d_w2.ap()[fc * 128 : (fc + 1) * 128, :]
                    )

                # ---- Phase D ----
                ctxD = tc.tile_pool(name="ppool", bufs=2, space="PSUM")
                ppool = ctxD.__enter__()
                ctxT = tc.tile_pool(name="tpool", bufs=2, space="PSUM")
                tpool = ctxT.__enter__()

                rn = [None] * 4  # pre-gain LN1 output per q-tile

                def proj_tile(i):
                    o, sz = QT[i]
                    ps = ppool.tile([128, H], F32, tag="proj", name="ps")
                    for n in range(2):
                        for k2 in range(NHC // 2):
                            nc.tensor.matmul(
                                ps[:sz, n * 512 : (n + 1) * 512],
                                attnT[:, 2 * k2 : 2 * k2 + 2, o : o + sz],
                                projw_sb[:, 2 * k2 : 2 * k2 + 2,
                                         n * 512 : (n + 1) * 512],
                                start=(k2 == 0),
                                stop=(k2 == NHC // 2 - 1),
                                perf_mode=DR,
                            )
                    # residual (xq already includes proj_b) + LN1;
                    # the *g gain-mult into y_sb is deferred to the FFN1
                    # window (DVE is idle there)
                    rn[i] = self_ln(nc, lpool, ps, xq_sb[:, i, :], sz, None,
                                    None, None, eps_t, in_scale=1.0 / SP8)

                def transpose_tile(i):
                    # yT holds the PRE-gain normalized rows; ln1_g is folded
                    # into w1 on the host.
                    o, sz = QT[i]
                    for k4 in range(2):
                        pt = tpool.tile([128, 4, 128], F32, tag="tr",
                                        name="pt")
                        for kk in range(4):
                            kc = 4 * k4 + kk
                            nc.tensor.transpose(
                                pt[:, kk, :sz],
                                rn[i][:sz, kc * 128 : (kc + 1) * 128],
                                ident[:sz, :sz],
                            )
                        nc.scalar.activation(
                            yT[:, 4 * k4 : 4 * k4 + 4, o : o + sz],
                            pt[:, 0:4, :sz],
                            AF.Copy,
                        )

                proj_tile(0)
                proj_tile(1)
                transpose_tile(0)
                proj_tile(2)
                transpose_tile(1)
                proj_tile(3)
                transpose_tile(2)
                transpose_tile(3)
                ctxT.__exit__(None, None, None)
                ctxD.__exit__(None, None, None)
                # deferred: y_sb = rn * ln1_g (E2 residual), overlaps FFN1
                for i, (o, sz) in enumerate(QT):
                    nc.vector.tensor_tensor(
                        out=y_sb[:sz, i, :], in0=rn[i][:sz],
                        in1=ln1g[:sz, :], op=ALU.mult,
                    )

                # ---- Phase E1: FFN1, two q-groups so the first group
                # starts as soon as transposes 0-1 are done ----
                with tc.tile_pool(name="hpool", bufs=2, space="PSUM") as hpool:
                    for qo, qw in ((0, 256), (256, 192)):
                        for f in range(NFT):
                            wsrc = w1a_sb if f < 16 else w1b_sb
                            fo = f if f < 16 else f - 16
                            ps = hpool.tile([128, 512], F32, tag="h",
                                            name="ps")
                            for kc in range(NHC):
                                nc.tensor.matmul(
                                    ps[:, :qw],
                                    wsrc[:, kc, fo * 128 : (fo + 1) * 128],
                                    yT[:, kc, qo : qo + qw],
                                    start=(kc == 0),
                                    stop=(kc == NHC - 1),
                                )
                            nc.scalar.activation(
                                hT[:, f, qo : qo + qw], ps[:, :qw], AF.Gelu,
                                bias=b1t[:, f : f + 1],
                            )

                # ---- Phase E2: FFN2, tile-serial ----
                with (
                    tc.tile_pool(name="zpool", bufs=2, space="PSUM") as zpool,
                ):
                    for i in range(4):
                        o, sz = QT[i]
                        zt = zpool.tile([128, H], F32, tag="z", name="zt")
                        for n in range(2):
                            for f2 in range(NFT // 2):
                                nc.tensor.matmul(
                                    zt[:sz, n * 512 : (n + 1) * 512],
                                    hT[:, 2 * f2 : 2 * f2 + 2, o : o + sz],
                                    w2_sb[:, 2 * f2 : 2 * f2 + 2,
                                          n * 512 : (n + 1) * 512],
                                    start=(f2 == 0),
                                    stop=False,
                                    perf_mode=DR,
                                )
                            # + (ffn_b2 + ln1_b) via a K=1 rank-one update
                            nc.tensor.matmul(
                                zt[:sz, n * 512 : (n + 1) * 512],
                                ones1[0:1, :sz],
                                b2row[0:1, n * 512 : (n + 1) * 512],
                                start=False,
                                stop=True,
                            )
                        # LN2 output overwrites y_sb[:, i, :] (residual
                        # already consumed); ln2_g/ln2_b applied on host
                        self_ln(nc, lpool, zt, y_sb[:, i, :], sz, None, None,
                                y_sb[:, i, :], eps_t, in_scale=1.0 / SW2)
                        nc.sync.dma_start(
                            d_out.ap()[o : o + sz, :], y_sb[:sz, i, :]
                        )

    nc.compile()
    return nc


def self_ln(nc, pool, ps, resid, sz, g_bc, b_bc, out_ap, eps_t,
            in_scale=1.0):
    """out = LayerNorm(ps * in_scale + resid) * g [+ b] over the free dim.

    ps is a PSUM tile [128, H] (clobbered as scratch); resid an SBUF tile.
    out_ap may alias resid. b_bc=None skips the bias add (folded upstream)."""
    r = pool.tile([128, H], F32, tag="r", name="r", bufs=4)
    sm = pool.tile([128, 1], F32, tag="sm", name="sm")
    # r = ps*in_scale + resid, sm = row sums (one fused op)
    nc.vector.scalar_tensor_tensor(
        out=r[:sz],
        in0=ps[:sz, :],
        scalar=in_scale,
        in1=resid[:sz],
        op0=ALU.mult,
        op1=ALU.add,
        accum_out=sm[:sz],
    )
    nm = pool.tile([128, 1], F32, tag="nm", name="nm")
    nc.vector.tensor_scalar_mul(nm[:sz], sm[:sz], -1.0 / H)
    # (r - mu)^2, accumulated into the variance sum; output is scratch and
    # clobbers the (already consumed) psum tile
    ssv = pool.tile([128, 1], F32, tag="ssv", name="ssv")
    nc.scalar.activation(
        ps[:sz, :], r[:sz], AF.Square, bias=nm[:sz, 0:1], accum_out=ssv[:sz]
    )
    sd = pool.tile([128, 1], F32, tag="sd", name="sd")
    nc.scalar.activation(sd[:sz], ssv[:sz], AF.Sqrt, scale=1.0 / H,
                         bias=eps_t[:sz])
    rstd = pool.tile([128, 1], F32, tag="rstd", name="rstd")
    nc.vector.reciprocal(rstd[:sz], sd[:sz])
    ts_out = out_ap if (g_bc is None and out_ap is not None) else r
    nc.vector.tensor_scalar(
        out=ts_out[:sz],
        in0=r[:sz],
        scalar1=nm[:sz],
        scalar2=rstd[:sz],
        op0=ALU.add,
        op1=ALU.mult,
    )
    if g_bc is None:
        pass  # caller applies gain/bias later (host or deferred)
    elif b_bc is None:
        nc.vector.tensor_tensor(out=out_ap[:sz], in0=r[:sz], in1=g_bc[:sz, :],
                                op=ALU.mult)
    else:
        nc.vector.tensor_tensor(out=r[:sz], in0=r[:sz], in1=g_bc[:sz, :],
                                op=ALU.mult)
        nc.vector.tensor_tensor(out=out_ap[:sz], in0=r[:sz],
                                in1=b_bc[:sz, :], op=ALU.add)
    return r


_NC = None


def _get_nc():
    global _NC
    if _NC is None:
        _NC = build_program()
    return _NC


def _prep_inputs(x, attn_bias, key_padding_mask, qkv_w, qkv_b, proj_w, proj_b,
                 ln1_g, ln1_b, ln2_g, ln2_b, ffn_w1, ffn_b1, ffn_w2, ffn_b2):
    bf = ml_dtypes.bfloat16
    f8 = ml_dtypes.float8_e4m3
    scale = HD ** -0.5
    qkv_ws = np.array(qkv_w, dtype=np.float32, copy=True)
    qkv_ws[:, :H] *= scale * SQ8
    qkv_ws[:, H : 2 * H] *= SK8
    qkv_ws[:, 2 * H :] *= SV8
    qkv_bs = np.array(qkv_b, dtype=np.float32, copy=True)
    qkv_bs[:H] *= scale
    qkb = np.empty((128, 16), np.float32)
    for m in range(8):
        qkb[:, m] = qkv_bs[m * 128 : (m + 1) * 128]
        qkb[:, 8 + m] = qkv_bs[H + m * 128 : H + (m + 1) * 128]
    ln1_b = np.asarray(ln1_b, np.float32)
    # ln1_b is folded out of the LN1 output: the FFN1 path gets it via
    # b1t (ln1_b @ w1), the LN2 residual path via b2row.
    w1 = np.asarray(ffn_w1, np.float32)
    b1t = (np.asarray(ffn_b1, np.float32)
           + ln1_b @ w1).reshape(NFT, 128).T.copy()
    w1eff = np.asarray(ln1_g, np.float32)[:, None] * w1
    b2row = (np.asarray(ffn_b2, np.float32) + ln1_b).reshape(1, H) * SW2
    shared = {
        "qkvw": qkv_ws.astype(f8),
        "qkb": qkb,
        "vb": qkv_bs[2 * H :].reshape(1, H).astype(np.float32),
        "projw": (np.asarray(proj_w, np.float32) * SP8).astype(f8),
        "w1": w1eff.astype(bf),
        "b1t": b1t,
        "w2": (np.asarray(ffn_w2, np.float32) * SW2).astype(f8),
        "b2row": b2row.astype(bf),
        "lnp": np.stack([ln1_g, ln1_b, ln2_g, ln2_b]).astype(np.float32),
    }
    in_maps = []
    x = np.asarray(x, dtype=np.float32)
    attn_bias = np.asarray(attn_bias, dtype=np.float32)
    proj_b = np.asarray(proj_b, dtype=np.float32)
    for c in range(8):
        b, half = c // 2, c % 2
        q0 = half * R
        # roll x columns so this core's own q rows occupy cols 0:448 of xT
        xv = x[b, :SV, :]          # [896, H]
        rolled = np.roll(xv, -q0, axis=0) if q0 else xv
        m = dict(shared)
        m["xT"] = np.ascontiguousarray(rolled.T).astype(f8)
        m["xq"] = (x[b, q0 : q0 + R, :] + proj_b[None, :]).astype(np.float32)
        # key axis must follow the same roll applied to xT's rows
        bT = np.ascontiguousarray(attn_bias[b, q0 : q0 + R, :SV].T)
        if q0:
            bT = np.roll(bT, -q0, axis=0)
        m["expbT"] = np.exp(bT).astype(bf)
        in_maps.append(m)
    return in_maps


def _assemble(results, g2, b2, dtype):
    out = np.zeros((B, S, H), dtype=np.float32)
    for c in range(8):
        b, half = c // 2, c % 2
        q0 = half * R
        out[b, q0 : q0 + R, :] = results[c]["out"] * g2 + b2
    return out.astype(dtype)


def kernel(**inputs):
    nc = _get_nc()
    in_maps = _prep_inputs(**inputs)
    res = run_bass_kernel_spmd(nc, in_maps, list(range(8)))
    g2 = np.asarray(inputs["ln2_g"], np.float32)[None, :]
    b2 = np.asarray(inputs["ln2_b"], np.float32)[None, :]
    return _assemble(res.results, g2, b2, np.asarray(inputs["x"]).dtype)


def kernel_profiled(inputs, tmpdir=None):
    nc = _get_nc()
    in_maps = _prep_inputs(**inputs)
    res = run_bass_kernel_spmd(
        nc, in_maps, list(range(8)), trace=True, tmpdir=tmpdir
    )
    g2 = np.asarray(inputs["ln2_g"], np.float32)[None, :]
    b2 = np.asarray(inputs["ln2_b"], np.float32)[None, :]
    return _assemble(res.results, g2, b2, np.float32), res


# revision 24
# speedup vs baseline: 1.0022x; 1.0022x over previous
"""Graphormer encoder layer on 8 trn2 NeuronCores.

Sharding: batch (4) x query-half (2) -> 8 cores, no collectives.
Core c handles batch b=c//2, query rows [q0, q0+448) with q0=(c%2)*448.
Only the first 896 sequence positions are computed (last 128 are padding:
keys are masked out and the reference zeroes those output rows, which the
host does during assembly).

Structure (single pass, PE kept dense):
- QKV and attention are software-pipelined per head-pair m: scores/exp for
  pair m run interleaved with attention-V of pair m-1 and the QKV chains of
  pair m+1, so the PE stays fed while the Activation engine does the exps.
- attn_bias enters softmax as exp(s+b) = exp(s)*exp(b); exp(b) is
  precomputed on the host, so no bias-add matmul on the PE.
- attention-V uses a [V_h | ones] stationary so one matmul pass yields both
  the weighted values (partitions 0-63) and the softmax denominator
  replicated on partitions 64-127.
- proj inputs and the first half of w1 are prefetched into the DMA-idle
  window during attention; the rest of w1 plus w2 stream during phase D/E1.
- FFN1 runs in two query groups so it starts right after the first two
  transposes; FFN2 runs tile-serial so LayerNorm2 of tile i overlaps the
  matmuls of tile i+1; ffn_b2+ln1_b ride an extra K=1 matmul row.
- ln1_g is folded into w1 on the host; ln2_g/ln2_b are applied on the host
  during assembly (both exact rewrites of the reference math).

Numerics: fp32 PSUM accumulation everywhere. QKV, proj, and FFN2 matmuls
run in fp8e4m3 with DoubleRow perf mode (weights pre-scaled x16/x64 on the
host, rescaled on psum read-out); attention scores/attention-V and FFN1
stay bf16. Measured end-to-end rel err 0.0168 vs the fp32 reference
(softmax and LayerNorm wash out the QKV/proj quantization; FFN2 fp8 is the
dominant, deliberate error term within the 2e-2 budget).
"""

import sys

sys.path.insert(0, "/opt/trn_rl_repo")

import numpy as np
import ml_dtypes

import concourse.bass as bass
import concourse.tile as tile
from concourse import bacc, mybir
from concourse.bass_utils import run_bass_kernel_spmd
from concourse.masks import make_identity

BF16 = mybir.dt.bfloat16
F32 = mybir.dt.float32
FP8 = mybir.dt.float8e4
AF = mybir.ActivationFunctionType
ALU = mybir.AluOpType
DR = mybir.MatmulPerfMode.DoubleRow
# fp8 weight pre-scales (host multiplies, kernel divides on psum read-out)
SQ8, SK8, SV8, SP8, SW2 = 64.0, 16.0, 16.0, 16.0, 16.0

B, S, H, NH, F = 4, 1024, 1024, 16, 4096
HD = H // NH          # 64
PAD = 128
SV = S - PAD          # 896 valid rows
R = SV // 2           # 448 query rows per core
NKT = SV // 128       # 7 k tiles
NHC = H // 128        # 8 chunks of H
NFT = F // 128        # 32 tiles of F
NP = NH // 2          # 8 head pairs
EPS = 1e-5
# q tiles within the 448 rows (last one ragged)
QT = [(0, 128), (128, 128), (256, 128), (384, 64)]


def free_bcast(ap2d, reps):
    """Insert a stride-0 dim after the partition dim: [P, W] -> [P, reps, W]."""
    return bass.AP(
        tensor=ap2d.tensor,
        offset=ap2d.offset,
        ap=[ap2d.ap[0], [0, reps]] + list(ap2d.ap[1:]),
    )


def bcast_row(dram_ap, offset_elems, row_len, nparts=128):
    """AP reading row_len dram elems replicated across nparts partitions."""
    base = dram_ap.ap()
    return bass.AP(
        tensor=base.tensor,
        offset=base.offset + offset_elems,
        ap=[[0, nparts], [1, row_len]],
    )


def build_program():
    nc = bacc.Bacc("TRN2", target_bir_lowering=False, debug=False)

    d_xT = nc.dram_tensor("xT", [H, SV], FP8, kind="ExternalInput")
    d_xq = nc.dram_tensor("xq", [R, H], F32, kind="ExternalInput")
    d_expbT = nc.dram_tensor("expbT", [SV, R], BF16, kind="ExternalInput")
    d_qkvw = nc.dram_tensor("qkvw", [H, 3 * H], FP8, kind="ExternalInput")
    d_qkb = nc.dram_tensor("qkb", [128, 16], F32, kind="ExternalInput")
    d_vb = nc.dram_tensor("vb", [1, H], F32, kind="ExternalInput")
    d_projw = nc.dram_tensor("projw", [H, H], FP8, kind="ExternalInput")
    d_w1 = nc.dram_tensor("w1", [H, F], BF16, kind="ExternalInput")
    d_b1t = nc.dram_tensor("b1t", [128, NFT], F32, kind="ExternalInput")
    d_w2 = nc.dram_tensor("w2", [F, H], FP8, kind="ExternalInput")
    d_b2row = nc.dram_tensor("b2row", [1, H], BF16, kind="ExternalInput")
    # rows: ln1_g, ln1_b, ln2_g, ln2_b
    d_lnp = nc.dram_tensor("lnp", [4, H], F32, kind="ExternalInput")
    d_out = nc.dram_tensor("out", [R, H], F32, kind="ExternalOutput")

    with tile.TileContext(nc) as tc:
        with (
            tc.tile_pool(name="const", bufs=1) as const,
            tc.tile_pool(name="gD", bufs=1) as gD,   # attnT + proj inputs
            tc.tile_pool(name="gW1a", bufs=1) as gW1a,  # first half of w1,
            # prefetched into the DMA-idle window during attention
        ):
            ident = const.tile([128, 128], F32)
            make_identity(nc, ident)
            eps_t = const.tile([128, 1], F32, tag="eps")
            nc.vector.memset(eps_t[:], EPS)
            ones1 = const.tile([1, 128], BF16, tag="ones1")
            nc.vector.memset(ones1[:], 1.0)
            qkb = const.tile([128, 16], F32, tag="qkb")
            nc.sync.dma_start(qkb[:], d_qkb.ap())
            b1t = const.tile([128, NFT], F32, tag="b1t")
            vb_bc = const.tile([128, H], F32, tag="vb")
            b2row = const.tile([1, H], BF16, tag="b2row")
            ln1g = const.tile([128, H], F32, tag="ln1g")
            b2e_bc = const.tile([128, H], F32, tag="b2e")

            attnT = gD.tile([128, NHC, R], FP8, tag="attnT")

            # -------- interleaved QKV (B) + attention (C) --------
            with (
                tc.tile_pool(name="gBC", bufs=1) as gBC,
                tc.tile_pool(name="pair", bufs=2) as pair,
                tc.tile_pool(name="psA", bufs=2, space="PSUM") as psA,
                tc.tile_pool(name="psS", bufs=2, space="PSUM") as psS,
                tc.tile_pool(name="psV", bufs=2, space="PSUM") as psV,
            ):
                qkvw_sb = gBC.tile([128, NHC, 3 * H], FP8, tag="qkvw")
                xT_sb = gBC.tile([128, NHC, SV], FP8, tag="xT")
                expb_sb = gBC.tile([128, NKT, R], BF16, tag="expb")

                def qkvw_cols(c0, w):
                    """DMA qkvw columns [c0, c0+w) into qkvw_sb (all kc)."""
                    nc.sync.dma_start(
                        qkvw_sb[:, :, c0 : c0 + w],
                        d_qkvw.ap()[:, c0 : c0 + w].rearrange(
                            "(kc p) c -> p kc c", p=128
                        ),
                    )

                # ordered so pair-0 chains start early
                qkvw_cols(0, 512)
                for kc in range(8):
                    nc.sync.dma_start(
                        xT_sb[:, kc, :], d_xT.ap()[kc * 128 : (kc + 1) * 128, :]
                    )
                qkvw_cols(H, 512)
                qkvw_cols(2 * H, 512)
                nc.sync.dma_start(vb_bc[:], bcast_row(d_vb, 0, H))
                nc.sync.dma_start(
                    expb_sb[:], d_expbT.ap().rearrange("(t p) q -> p t q", p=128)
                )
                qkvw_cols(512, 512)
                qkvw_cols(H + 512, 512)
                qkvw_cols(2 * H + 512, 512)

                # prefetch phase-D inputs (issued now; lands during attention)
                xq_sb = gD.tile([128, 4, H], F32, tag="xq")
                for i, (o, sz) in enumerate(QT):
                    nc.sync.dma_start(xq_sb[:sz, i, :], d_xq.ap()[o : o + sz, :])
                projw_sb = gD.tile([128, NHC, H], FP8, tag="projw")
                for kc in range(NHC):
                    nc.sync.dma_start(
                        projw_sb[:, kc, :],
                        d_projw.ap()[kc * 128 : (kc + 1) * 128, :],
                    )
                nc.sync.dma_start(ln1g[:], bcast_row(d_lnp, 0, H))
                nc.sync.dma_start(b1t[:], d_b1t.ap())
                nc.sync.dma_start(b2e_bc[:], bcast_row(d_lnp, H, H))
                w1a_sb = gW1a.tile([128, NHC, F // 2], BF16, tag="w1a")
                for c in range(4):
                    nc.sync.dma_start(
                        w1a_sb[:, :, c * 512 : (c + 1) * 512],
                        d_w1.ap()[:, c * 512 : (c + 1) * 512].rearrange(
                            "(kc p) f -> p kc f", p=128
                        ),
                    )

                prev = None  # (m, E0, E1, vnat) awaiting attention-V

                def attn_v_j(m, E, vnat, j):
                    """attention-V + divide for (pair m, head slot j)."""
                    pv = psV.tile([128, 512], F32, tag="pv", name="pv")
                    for t in range(NKT):
                        nc.tensor.matmul(
                            pv[:, :R],
                            vnat[:, t, j, :],
                            E[:, t, :],
                            start=(t == 0),
                            stop=(t == NKT - 1),
                        )
                    rec = pair.tile([128, R], F32, tag=f"rec{j}",
                                    name=f"rec{j}")
                    nc.vector.reciprocal(rec[64:128, :], pv[64:128, :R])
                    nc.vector.tensor_tensor(
                        out=attnT[64 * j : 64 * j + 64, m, :],
                        in0=pv[0:64, :R],
                        in1=rec[64:128, :],
                        op=ALU.mult,
                    )

                def qkv_pair(m):
                    """QKV chains for pair m; returns (qTm, kTm, vnat)."""
                    qTm = pair.tile([128, R], BF16, tag="qTm", name="qTm")
                    kTm = pair.tile([128, SV], BF16, tag="kTm", name="kTm")
                    vnat = pair.tile([128, NKT, 2, 128], BF16, tag="vnat",
                                     name="vnat")
                    # ones blocks for the fused attention-V row sums
                    nc.gpsimd.memset(vnat[:, :, :, 64:128], 1.0)

                    ps = psA.tile([128, 512], F32, tag="qkv", name="ps")
                    for k2 in range(NHC // 2):
                        nc.tensor.matmul(
                            ps[:, :R],
                            qkvw_sb[:, 2 * k2 : 2 * k2 + 2,
                                    m * 128 : (m + 1) * 128],
                            xT_sb[:, 2 * k2 : 2 * k2 + 2, 0:R],
                            start=(k2 == 0),
                            stop=(k2 == NHC // 2 - 1),
                            perf_mode=DR,
                        )
                    nc.vector.tensor_scalar(
                        out=qTm[:], in0=ps[:, :R],
                        scalar1=1.0 / SQ8, scalar2=qkb[:, m : m + 1],
                        op0=ALU.mult, op1=ALU.add,
                    )
                    for n in range(2):
                        ps = psA.tile([128, 512], F32, tag="qkv", name="ps")
                        for k2 in range(NHC // 2):
                            nc.tensor.matmul(
                                ps[:, :R],
                                qkvw_sb[:, 2 * k2 : 2 * k2 + 2,
                                        H + m * 128 : H + (m + 1) * 128],
                                xT_sb[:, 2 * k2 : 2 * k2 + 2,
                                      n * R : (n + 1) * R],
                                start=(k2 == 0),
                                stop=(k2 == NHC // 2 - 1),
                                perf_mode=DR,
                            )
                        if n == 0:
                            nc.vector.tensor_scalar(
                                out=kTm[:, n * R : (n + 1) * R], in0=ps[:, :R],
                                scalar1=1.0 / SK8, scalar2=qkb[:, 8 + m : 9 + m],
                                op0=ALU.mult, op1=ALU.add,
                            )
                        else:
                            nc.scalar.activation(
                                kTm[:, n * R : (n + 1) * R], ps[:, :R],
                                AF.Identity, scale=1.0 / SK8,
                                bias=qkb[:, 8 + m : 9 + m],
                            )
                    for t0 in range(0, NKT, 4):
                        tw = min(4, NKT - t0)
                        ps = psA.tile([128, 4, 128], F32, tag="qkv", name="ps")
                        for t in range(t0, t0 + tw):
                            for k2 in range(NHC // 2):
                                nc.tensor.matmul(
                                    ps[:, t - t0, :],
                                    xT_sb[:, 2 * k2 : 2 * k2 + 2,
                                          t * 128 : (t + 1) * 128],
                                    qkvw_sb[:, 2 * k2 : 2 * k2 + 2,
                                            2 * H + m * 128 : 2 * H + (m + 1) * 128],
                                    start=(k2 == 0),
                                    stop=(k2 == NHC // 2 - 1),
                                    perf_mode=DR,
                                )
                        nc.vector.scalar_tensor_tensor(
                            out=vnat[:, t0 : t0 + tw, 0:2, 0:64],
                            in0=ps[:, 0:tw, :],
                            scalar=1.0 / SV8,
                            in1=free_bcast(vb_bc[:, m * 128 : (m + 1) * 128], tw),
                            op0=ALU.mult,
                            op1=ALU.add,
                        )
                    return qTm, kTm, vnat

                cur = qkv_pair(0)
                for m in range(NP):
                    qTm, kTm, vnat = cur
                    # scores + exp for pair m; the previous pair's
                    # attention-V and the NEXT pair's QKV are interleaved
                    # to keep the PE fed during the exp waits
                    Es = [
                        pair.tile([128, NKT, R], BF16, tag="E0", name="E0"),
                        pair.tile([128, NKT, R], BF16, tag="E1", name="E1"),
                    ]

                    def sc_batch(j, t0):
                        po = 64 * j
                        tw = min(2, NKT - t0)
                        sc = psS.tile([128, 2, 512], F32, tag="sc", name="sc")
                        for t in range(t0, t0 + tw):
                            nc.tensor.matmul(
                                sc[:, t - t0, :R],
                                kTm[po : po + 64, t * 128 : (t + 1) * 128],
                                qTm[po : po + 64, :],
                                start=True,
                                stop=True,
                            )
                        nc.scalar.activation(
                            Es[j][:, t0 : t0 + tw, :], sc[:, 0:tw, 0:R], AF.Exp
                        )

                    def expb_mult(j):
                        # fold in exp(attn_bias) (host-precomputed)
                        nc.vector.tensor_tensor(
                            out=Es[j][:, :, :], in0=Es[j][:, :, :],
                            in1=expb_sb[:, :, :], op=ALU.mult,
                        )

                    sc_batch(0, 0)
                    sc_batch(0, 2)
                    if prev is not None:
                        attn_v_j(prev[0], prev[1][0], prev[2], 0)
                    sc_batch(0, 4)
                    sc_batch(0, 6)
                    expb_mult(0)
                    if prev is not None:
                        attn_v_j(prev[0], prev[1][1], prev[2], 1)
                    if m < NP - 1:
                        cur = qkv_pair(m + 1)
                    sc_batch(1, 0)
                    sc_batch(1, 2)
                    sc_batch(1, 4)
                    sc_batch(1, 6)
                    expb_mult(1)

                    prev = (m, Es, vnat)

                attn_v_j(prev[0], prev[1][0], prev[2], 0)
                attn_v_j(prev[0], prev[1][1], prev[2], 1)

            # -------- proj + LN1 + transpose (D), FFN (E) --------
            with (
                tc.tile_pool(name="gW", bufs=1) as gW,
                tc.tile_pool(name="lpool", bufs=2) as lpool,
            ):
                y_sb = gW.tile([128, 4, H], F32, tag="y")
                yT = gW.tile([128, NHC, R], BF16, tag="yT")
                hT = gW.tile([128, NFT, R], FP8, tag="hT")
                w1b_sb = gW.tile([128, NHC, F // 2], BF16, tag="w1b")
                w2_sb = gW.tile([128, NFT, H], FP8, tag="w2")
                for c in range(4):
                    nc.sync.dma_start(
                        w1b_sb[:, :, c * 512 : (c + 1) * 512],
                        d_w1.ap()[:, 2048 + c * 512 : 2048 + (c + 1) * 512]
                        .rearrange("(kc p) f -> p kc f", p=128),
                    )
                for fc in range(NFT):
                    nc.sync.dma_start(
                        w2_sb[:, fc, :], d_w2.ap()[fc * 128 : (fc + 1) * 128, :]
                    )

                # ---- Phase D ----
                ctxD = tc.tile_pool(name="ppool", bufs=2, space="PSUM")
                ppool = ctxD.__enter__()
                ctxT = tc.tile_pool(name="tpool", bufs=2, space="PSUM")
                tpool = ctxT.__enter__()

                rn = [None] * 4  # pre-gain LN1 output per q-tile

                def proj_tile(i):
                    o, sz = QT[i]
                    ps = ppool.tile([128, H], F32, tag="proj", name="ps")
                    for n in range(2):
                        for k2 in range(NHC // 2):
                            nc.tensor.matmul(
                                ps[:sz, n * 512 : (n + 1) * 512],
                                attnT[:, 2 * k2 : 2 * k2 + 2, o : o + sz],
                                projw_sb[:, 2 * k2 : 2 * k2 + 2,
                                         n * 512 : (n + 1) * 512],
                                start=(k2 == 0),
                                stop=(k2 == NHC // 2 - 1),
                                perf_mode=DR,
                            )
                    # residual (xq already includes proj_b) + LN1;
                    # the *g gain-mult into y_sb is deferred to the FFN1
                    # window (DVE is idle there)
                    rn[i] = self_ln(nc, lpool, ps, xq_sb[:, i, :], sz, None,
                                    None, None, eps_t, in_scale=1.0 / SP8)

                def transpose_tile(i):
                    # yT holds the PRE-gain normalized rows; ln1_g is folded
                    # into w1 on the host.
                    o, sz = QT[i]
                    for k4 in range(2):
                        pt = tpool.tile([128, 4, 128], F32, tag="tr",
                                        name="pt")
                        for kk in range(4):
                            kc = 4 * k4 + kk
                            nc.tensor.transpose(
                                pt[:, kk, :sz],
                                rn[i][:sz, kc * 128 : (kc + 1) * 128],
                                ident[:sz, :sz],
                            )
                        nc.scalar.activation(
                            yT[:, 4 * k4 : 4 * k4 + 4, o : o + sz],
                            pt[:, 0:4, :sz],
                            AF.Copy,
                        )

                proj_tile(0)
                proj_tile(1)
                transpose_tile(0)
                proj_tile(2)
                transpose_tile(1)
                proj_tile(3)
                transpose_tile(2)
                transpose_tile(3)
                ctxT.__exit__(None, None, None)
                ctxD.__exit__(None, None, None)
                # deferred E2-residual prep (DVE idles during FFN1):
                # y_sb = rn * ln1_g + (ffn_b2 + ln1_b), so FFN2 needs no
                # bias-row matmul
                for i, (o, sz) in enumerate(QT):
                    nc.vector.tensor_tensor(
                        out=y_sb[:sz, i, :], in0=rn[i][:sz],
                        in1=ln1g[:sz, :], op=ALU.mult,
                    )
                    nc.vector.tensor_tensor(
                        out=y_sb[:sz, i, :], in0=y_sb[:sz, i, :],
                        in1=b2e_bc[:sz, :], op=ALU.add,
                    )

                # ---- Phase E1: FFN1, two q-groups so the first group
                # starts as soon as transposes 0-1 are done ----
                with tc.tile_pool(name="hpool", bufs=2, space="PSUM") as hpool:
                    for qo, qw in ((0, 256), (256, 192)):
                        for f in range(NFT):
                            wsrc = w1a_sb if f < 16 else w1b_sb
                            fo = f if f < 16 else f - 16
                            ps = hpool.tile([128, 512], F32, tag="h",
                                            name="ps")
                            for kc in range(NHC):
                                nc.tensor.matmul(
                                    ps[:, :qw],
                                    wsrc[:, kc, fo * 128 : (fo + 1) * 128],
                                    yT[:, kc, qo : qo + qw],
                                    start=(kc == 0),
                                    stop=(kc == NHC - 1),
                                )
                            nc.scalar.activation(
                                hT[:, f, qo : qo + qw], ps[:, :qw], AF.Gelu,
                                bias=b1t[:, f : f + 1],
                            )

                # ---- Phase E2: FFN2, tile-serial ----
                with (
                    tc.tile_pool(name="zpool", bufs=2, space="PSUM") as zpool,
                ):
                    for i in range(4):
                        o, sz = QT[i]
                        zt = zpool.tile([128, H], F32, tag="z", name="zt")
                        for n in range(2):
                            for f2 in range(NFT // 2):
                                nc.tensor.matmul(
                                    zt[:sz, n * 512 : (n + 1) * 512],
                                    hT[:, 2 * f2 : 2 * f2 + 2, o : o + sz],
                                    w2_sb[:, 2 * f2 : 2 * f2 + 2,
                                          n * 512 : (n + 1) * 512],
                                    start=(f2 == 0),
                                    stop=(f2 == NFT // 2 - 1),
                                    perf_mode=DR,
                                )
                        # LN2 output overwrites y_sb[:, i, :] (residual
                        # already consumed); ln2_g/ln2_b applied on host
                        self_ln(nc, lpool, zt, y_sb[:, i, :], sz, None, None,
                                y_sb[:, i, :], eps_t, in_scale=1.0 / SW2)
                        nc.sync.dma_start(
                            d_out.ap()[o : o + sz, :], y_sb[:sz, i, :]
                        )

    nc.compile()
    return nc


def self_ln(nc, pool, ps, resid, sz, g_bc, b_bc, out_ap, eps_t,
            in_scale=1.0):
    """out = LayerNorm(ps * in_scale + resid) * g [+ b] over the free dim.

    ps is a PSUM tile [128, H] (clobbered as scratch); resid an SBUF tile.
    out_ap may alias resid. b_bc=None skips the bias add (folded upstream)."""
    r = pool.tile([128, H], F32, tag="r", name="r", bufs=4)
    sm = pool.tile([128, 1], F32, tag="sm", name="sm")
    # r = ps*in_scale + resid, sm = row sums (one fused op)
    nc.vector.scalar_tensor_tensor(
        out=r[:sz],
        in0=ps[:sz, :],
        scalar=in_scale,
        in1=resid[:sz],
        op0=ALU.mult,
        op1=ALU.add,
        accum_out=sm[:sz],
    )
    nm = pool.tile([128, 1], F32, tag="nm", name="nm")
    nc.vector.tensor_scalar_mul(nm[:sz], sm[:sz], -1.0 / H)
    # (r - mu)^2, accumulated into the variance sum; output is scratch and
    # clobbers the (already consumed) psum tile
    ssv = pool.tile([128, 1], F32, tag="ssv", name="ssv")
    nc.scalar.activation(
        ps[:sz, :], r[:sz], AF.Square, bias=nm[:sz, 0:1], accum_out=ssv[:sz]
    )
    sd = pool.tile([128, 1], F32, tag="sd", name="sd")
    nc.scalar.activation(sd[:sz], ssv[:sz], AF.Sqrt, scale=1.0 / H,
                         bias=eps_t[:sz])
    rstd = pool.tile([128, 1], F32, tag="rstd", name="rstd")
    nc.vector.reciprocal(rstd[:sz], sd[:sz])
    ts_out = out_ap if (g_bc is None and out_ap is not None) else r
    nc.vector.tensor_scalar(
        out=ts_out[:sz],
        in0=r[:sz],
        scalar1=nm[:sz],
        scalar2=rstd[:sz],
        op0=ALU.add,
        op1=ALU.mult,
    )
    if g_bc is None:
        pass  # caller applies gain/bias later (host or deferred)
    elif b_bc is None:
        nc.vector.tensor_tensor(out=out_ap[:sz], in0=r[:sz], in1=g_bc[:sz, :],
                                op=ALU.mult)
    else:
        nc.vector.tensor_tensor(out=r[:sz], in0=r[:sz], in1=g_bc[:sz, :],
                                op=ALU.mult)
        nc.vector.tensor_tensor(out=out_ap[:sz], in0=r[:sz],
                                in1=b_bc[:sz, :], op=ALU.add)
    return r


_NC = None


def _get_nc():
    global _NC
    if _NC is None:
        _NC = build_program()
    return _NC


def _prep_inputs(x, attn_bias, key_padding_mask, qkv_w, qkv_b, proj_w, proj_b,
                 ln1_g, ln1_b, ln2_g, ln2_b, ffn_w1, ffn_b1, ffn_w2, ffn_b2):
    bf = ml_dtypes.bfloat16
    f8 = ml_dtypes.float8_e4m3
    scale = HD ** -0.5
    qkv_ws = np.array(qkv_w, dtype=np.float32, copy=True)
    qkv_ws[:, :H] *= scale * SQ8
    qkv_ws[:, H : 2 * H] *= SK8
    qkv_ws[:, 2 * H :] *= SV8
    qkv_bs = np.array(qkv_b, dtype=np.float32, copy=True)
    qkv_bs[:H] *= scale
    qkb = np.empty((128, 16), np.float32)
    for m in range(8):
        qkb[:, m] = qkv_bs[m * 128 : (m + 1) * 128]
        qkb[:, 8 + m] = qkv_bs[H + m * 128 : H + (m + 1) * 128]
    ln1_b = np.asarray(ln1_b, np.float32)
    # ln1_b is folded out of the LN1 output: the FFN1 path gets it via
    # b1t (ln1_b @ w1), the LN2 residual path via b2row.
    w1 = np.asarray(ffn_w1, np.float32)
    b1t = (np.asarray(ffn_b1, np.float32)
           + ln1_b @ w1).reshape(NFT, 128).T.copy()
    w1eff = np.asarray(ln1_g, np.float32)[:, None] * w1
    b2row = (np.asarray(ffn_b2, np.float32) + ln1_b).reshape(1, H) * SW2
    shared = {
        "qkvw": qkv_ws.astype(f8),
        "qkb": qkb,
        "vb": qkv_bs[2 * H :].reshape(1, H).astype(np.float32),
        "projw": (np.asarray(proj_w, np.float32) * SP8).astype(f8),
        "w1": w1eff.astype(bf),
        "b1t": b1t,
        "w2": (np.asarray(ffn_w2, np.float32) * SW2).astype(f8),
        "b2row": b2row.astype(bf),
        "lnp": np.stack(
            [ln1_g, np.asarray(ffn_b2, np.float32) + ln1_b, ln2_g, ln2_b]
        ).astype(np.float32),
    }
    in_maps = []
    x = np.asarray(x, dtype=np.float32)
    attn_bias = np.asarray(attn_bias, dtype=np.float32)
    proj_b = np.asarray(proj_b, dtype=np.float32)
    for c in range(8):
        b, half = c // 2, c % 2
        q0 = half * R
        # roll x columns so this core's own q rows occupy cols 0:448 of xT
        xv = x[b, :SV, :]          # [896, H]
        rolled = np.roll(xv, -q0, axis=0) if q0 else xv
        m = dict(shared)
        m["xT"] = np.ascontiguousarray(rolled.T).astype(f8)
        m["xq"] = (x[b, q0 : q0 + R, :] + proj_b[None, :]).astype(np.float32)
        # key axis must follow the same roll applied to xT's rows
        bT = np.ascontiguousarray(attn_bias[b, q0 : q0 + R, :SV].T)
        if q0:
            bT = np.roll(bT, -q0, axis=0)
        m["expbT"] = np.exp(bT).astype(bf)
        in_maps.append(m)
    return in_maps


def _assemble(results, g2, b2, dtype):
    out = np.zeros((B, S, H), dtype=np.float32)
    for c in range(8):
        b, half = c // 2, c % 2
        q0 = half * R
        out[b, q0 : q0 + R, :] = results[c]["out"] * g2 + b2
    return out.astype(dtype)


def kernel(**inputs):
    nc = _get_nc()
    in_maps = _prep_inputs(**inputs)
    res = run_bass_kernel_spmd(nc, in_maps, list(range(8)))
    g2 = np.asarray(inputs["ln2_g"], np.float32)[None, :]
    b2 = np.asarray(inputs["ln2_b"], np.float32)[None, :]
    return _assemble(res.results, g2, b2, np.asarray(inputs["x"]).dtype)


def kernel_profiled(inputs, tmpdir=None):
    nc = _get_nc()
    in_maps = _prep_inputs(**inputs)
    res = run_bass_kernel_spmd(
        nc, in_maps, list(range(8)), trace=True, tmpdir=tmpdir
    )
    g2 = np.asarray(inputs["ln2_g"], np.float32)[None, :]
    b2 = np.asarray(inputs["ln2_b"], np.float32)[None, :]
    return _assemble(res.results, g2, b2, np.float32), res
